# revision 1
# baseline (speedup 1.0000x reference)
"""Trainium2 Bass kernel for nn_EquivariantGNN_GAT (2-layer GAT + linear + mean pool).

Strategy (8 NeuronCores, SPMD single program):
  - Nodes padded to 50176 = 392 blocks of 128; each core owns 49 dst-blocks
    (6272 nodes) and all edges incident (by dst) on them, host-sorted by dst.
  - Per layer, each core computes hs = x @ [W | 0 | W@a_src | W@a_dst] for its
    node shard in f32, stores the per-node row [h(128) | 1 | s_src | s_dst]
    cast to bf16 (512B rows), AllGathers the full [50176, 256] bf16 table
    into HBM, then processes its edges in chunks of 128 via dma_gather of
    hs[src] rows. int16 gather indices are handled by splitting each block's
    edges into src<32768 ("lo") and src>=32768 ("hi") halves gathered from
    offset table views; gathers are capped at 8 chunks (1024 descriptors)
    to fit the SWDGE ring.
  - Per chunk: one-hot dst matrix scaled by exp(leaky_relu(s_src + s_dst))
    built on DVE (scalar_tensor_tensor with fused accum for the s_dst
    expansion), then a single bf16 matmul accumulates numerator + softmax
    denominator ([h | 1] columns) into f32 PSUM per dst block.
  - Softmax max-subtraction is skipped (mathematically equivalent here).
  - Final: y = x3 @ Wlin + blin per block, per-graph mean pool via one-hot
    matmul accumulated in PSUM, AllReduce over cores, scale by 1/counts.

kernel(**inputs) takes the FULL problem inputs and returns the [64, 32] output.
"""
import sys

sys.path.insert(0, "/opt/trn_rl_repo")

import ml_dtypes
import numpy as np

import concourse.bass as bass
import concourse.bacc as bacc
import concourse.mybir as mybir
import concourse.tile as tile
import concourse.bass_utils as bass_utils
from concourse.bass import IndirectOffsetOnAxis
from concourse.bass_interp import get_hw_module

N = 50000
E = 1600000
H = 128
O = 32
T = 100
G = 64
P = 128
NCORES = 8
NBPC = 49              # dst blocks per core
NB = NBPC * NCORES     # 392 blocks -> 50176 padded nodes
NPAD = NB * P
SH = NBPC * P          # 6272 nodes per core
ROW = 256              # bf16 elems per hs row: [h(128) | 1 | ssrc | sdst | 0pad]
WCOL = 131             # computed columns: [W(128) | 0 | W@a_s | W@a_d]
LO = 32768             # int16 index limit; src >= LO gathered from offset view
GMAX = 8               # chunks per dma_gather (1024 descs fit the SWDGE ring)
NEG = 0.2

F32 = mybir.dt.float32
BF16 = mybir.dt.bfloat16
I32 = mybir.dt.int32
I16 = mybir.dt.int16
ALU = mybir.AluOpType
AF = mybir.ActivationFunctionType
NPBF = ml_dtypes.bfloat16


# ---------------------------------------------------------------- host prep
def _wrap16(flat):
    """dma_gather index layout: idx k -> [k%16, k//16], replicated x8."""
    n = flat.shape[0]
    assert n % 16 == 0
    w = flat.reshape(n // 16, 16).T          # [16, n//16]
    return np.tile(w, (8, 1))                 # [128, n//16]


def _prep(inputs):
    pos = np.ascontiguousarray(np.asarray(inputs["pos"], np.float32))
    z = np.asarray(inputs["z"]).astype(np.int64)
    ei = np.asarray(inputs["edge_index"]).astype(np.int64)
    batch = np.asarray(inputs["batch"]).astype(np.int64)

    loop = np.arange(N, dtype=np.int64)
    src = np.concatenate([ei[0], loop])
    dst = np.concatenate([ei[1], loop])
    order = np.argsort(dst, kind="stable")
    src_s = src[order].astype(np.int32)
    dst_s = dst[order].astype(np.int32)

    bnd = np.searchsorted(dst_s, np.arange(NB + 1) * P).astype(np.int64)

    block_lo = {}
    block_hi = {}
    for b in range(NB):
        e0, e1 = int(bnd[b]), int(bnd[b + 1])
        s = src_s[e0:e1]
        d = dst_s[e0:e1] - b * P
        m = s < LO
        block_lo[b] = (s[m], d[m])
        block_hi[b] = (s[~m] - LO, d[~m])

    nlo = np.zeros(NBPC, np.int64)
    nhi = np.zeros(NBPC, np.int64)
    for i in range(NBPC):
        for c in range(NCORES):
            b = NBPC * c + i
            nlo[i] = max(nlo[i], (len(block_lo[b][0]) + P - 1) // P)
            nhi[i] = max(nhi[i], (len(block_hi[b][0]) + P - 1) // P)
        if nlo[i] + nhi[i] == 0:
            nhi[i] = 1
    CH = nlo + nhi
    off = np.zeros(NBPC + 1, np.int64)
    off[1:] = np.cumsum(CH)
    TOTCH = int(off[-1])

    idx_w = np.zeros((NCORES, P, 8 * TOTCH), np.int16)
    dstrel = np.full((NCORES, P, TOTCH), -1.0, NPBF)
    zidx = np.zeros((NCORES, P, NBPC), np.int32)
    batchrel = np.full((NCORES, P, NBPC), -1.0, np.float32)
    pos_shard = np.zeros((NCORES, SH, 3), np.float32)

    for c in range(NCORES):
        for i in range(NBPC):
            b = NBPC * c + i
            for (s, d), n_ch, cbase in (
                (block_lo[b], int(nlo[i]), int(off[i])),
                (block_hi[b], int(nhi[i]), int(off[i]) + int(nlo[i])),
            ):
                if n_ch == 0:
                    continue
                ne = len(s)
                flat = np.zeros(n_ch * P, np.int16)
                flat[:ne] = s.astype(np.int16)
                dr = np.full(n_ch * P, -1.0, np.float32)
                dr[:ne] = d.astype(np.float32)
                idx_w[c, :, 8 * cbase:8 * (cbase + n_ch)] = _wrap16(flat)
                dstrel[c, :, cbase:cbase + n_ch] = \
                    dr.reshape(n_ch, P).T.astype(NPBF)
        n0 = SH * c
        nodes = np.arange(n0, n0 + SH)
        valid = nodes < N
        zc = np.zeros(SH, np.int32)
        zc[valid] = z[nodes[valid]].astype(np.int32)
        zidx[c] = zc.reshape(NBPC, P).T
        bc = np.full(SH, -1.0, np.float32)
        bc[valid] = batch[nodes[valid]].astype(np.float32)
        batchrel[c] = bc.reshape(NBPC, P).T
        pos_shard[c][valid] = pos[nodes[valid]]

    counts = np.bincount(batch, minlength=G).astype(np.float32)
    cinv = (1.0 / np.maximum(counts, 1.0)).astype(np.float32).reshape(G, 1)

    iota_f = np.tile(np.arange(P, dtype=np.float32), (P, 1))
    consts = dict(
        iota=np.ascontiguousarray(iota_f),
        iotab=np.ascontiguousarray(iota_f.astype(NPBF)),
        ident=np.eye(P, dtype=np.float32),
        ones1=np.ones((1, P), NPBF),
        W1=np.ascontiguousarray(np.asarray(inputs["W1"], np.float32)),
        W1T=np.ascontiguousarray(np.asarray(inputs["W1"], np.float32).T),
        a1s=np.asarray(inputs["a1_src"], np.float32).reshape(H, 1),
        a1d=np.asarray(inputs["a1_dst"], np.float32).reshape(H, 1),
        b1rep=np.ascontiguousarray(
            np.tile(np.asarray(inputs["b1"], np.float32), (P, 1))),
        W2=np.ascontiguousarray(np.asarray(inputs["W2"], np.float32)),
        W2T=np.ascontiguousarray(np.asarray(inputs["W2"], np.float32).T),
        a2s=np.asarray(inputs["a2_src"], np.float32).reshape(H, 1),
        a2d=np.asarray(inputs["a2_dst"], np.float32).reshape(H, 1),
        b2rep=np.ascontiguousarray(
            np.tile(np.asarray(inputs["b2"], np.float32), (P, 1))),
        Wlin=np.ascontiguousarray(np.asarray(inputs["Wlin"], np.float32)),
        blinrep=np.ascontiguousarray(
            np.tile(np.asarray(inputs["blin"], np.float32), (P, 1))),
        emb=np.ascontiguousarray(np.asarray(inputs["emb"], np.float32)),
        cinv=cinv,
    )

    meta = dict(nlo=tuple(int(x) for x in nlo),
                nhi=tuple(int(x) for x in nhi),
                off=tuple(int(x) for x in off),
                TOTCH=TOTCH)
    percore = dict(idx_w=idx_w, dstrel=dstrel, zidx=zidx,
                   batchrel=batchrel, pos_shard=pos_shard)
    return meta, percore, consts


# ---------------------------------------------------------------- program
def _build(meta, analysis=False, gmax=GMAX, nqueues=1, skip=(),
           scratch=16384, gbufs=3, mtact=False, bufboost=0):
    nlo = meta["nlo"]
    nhi = meta["nhi"]
    off = meta["off"]
    TOTCH = meta["TOTCH"]
    qstate = {"q": 0}

    def next_q():
        q = qstate["q"]
        qstate["q"] = (q + 1) % nqueues
        return q

    nc = bacc.Bacc("TRN2", target_bir_lowering=False, debug=False,
                   enable_asserts=False,
                   num_devices=1 if analysis else NCORES,
                   num_swdge_queues=nqueues,
                   dynamic_dma_scratch_size=scratch)

    t_idx = nc.dram_tensor("idx_w", [P, 8 * TOTCH], I16, kind="ExternalInput")
    t_dstr = nc.dram_tensor("dstrel", [P, TOTCH], BF16, kind="ExternalInput")
    t_zidx = nc.dram_tensor("zidx", [P, NBPC], I32, kind="ExternalInput")
    t_brel = nc.dram_tensor("batchrel", [P, NBPC], F32, kind="ExternalInput")
    t_pos = nc.dram_tensor("pos_shard", [SH, 3], F32, kind="ExternalInput")
    t_emb = nc.dram_tensor("emb", [T, 125], F32, kind="ExternalInput")
    t_iota = nc.dram_tensor("iota", [P, P], F32, kind="ExternalInput")
    t_iotab = nc.dram_tensor("iotab", [P, P], BF16, kind="ExternalInput")
    t_ident = nc.dram_tensor("ident", [P, P], F32, kind="ExternalInput")
    t_ones1 = nc.dram_tensor("ones1", [1, P], BF16, kind="ExternalInput")
    t_W1 = nc.dram_tensor("W1", [H, H], F32, kind="ExternalInput")
    t_W1T = nc.dram_tensor("W1T", [H, H], F32, kind="ExternalInput")
    t_a1s = nc.dram_tensor("a1s", [H, 1], F32, kind="ExternalInput")
    t_a1d = nc.dram_tensor("a1d", [H, 1], F32, kind="ExternalInput")
    t_b1rep = nc.dram_tensor("b1rep", [P, H], F32, kind="ExternalInput")
    t_W2 = nc.dram_tensor("W2", [H, H], F32, kind="ExternalInput")
    t_W2T = nc.dram_tensor("W2T", [H, H], F32, kind="ExternalInput")
    t_a2s = nc.dram_tensor("a2s", [H, 1], F32, kind="ExternalInput")
    t_a2d = nc.dram_tensor("a2d", [H, 1], F32, kind="ExternalInput")
    t_b2rep = nc.dram_tensor("b2rep", [P, H], F32, kind="ExternalInput")
    t_Wlin = nc.dram_tensor("Wlin", [H, O], F32, kind="ExternalInput")
    t_blinrep = nc.dram_tensor("blinrep", [P, O], F32, kind="ExternalInput")
    t_cinv = nc.dram_tensor("cinv", [G, 1], F32, kind="ExternalInput")
    t_out = nc.dram_tensor("out", [G, O], F32, kind="ExternalOutput")

    groups = [list(range(NCORES))]

    with tile.TileContext(nc) as tc:
        with (
            tc.tile_pool(name="const", bufs=1) as cpool,
            tc.tile_pool(name="work", bufs=3) as wpool,
            tc.tile_pool(name="gat", bufs=gbufs) as gpool,
            tc.tile_pool(name="mt", bufs=14 + bufboost) as mpool,
            tc.tile_pool(name="gp", bufs=6 + bufboost) as gppool,
            tc.tile_pool(name="sml", bufs=4 + bufboost) as spool,
            tc.tile_pool(name="ps", bufs=2, space="PSUM") as pspool,
            tc.tile_pool(name="acc", bufs=1, space="PSUM") as apool,
            tc.tile_pool(name="dram", bufs=1, space="DRAM") as dpool,
        ):
            def cload(nm, t, shape, dtype=F32):
                tl = cpool.tile(shape, dtype, name=nm, tag=nm)
                nc.sync.dma_start(tl[:, :], t.ap())
                return tl

            iota_sb = cload("iota_sb", t_iota, [P, P])
            iotab_sb = cload("iotab_sb", t_iotab, [P, P], BF16)
            ident_sb = cload("ident_sb", t_ident, [P, P])
            ones1_sb = cload("ones1_sb", t_ones1, [1, P], BF16)
            eps_sb = cpool.tile([P, 1], F32, name="eps_sb", tag="eps_sb")
            nc.vector.memset(eps_sb[:, :], 1e-38)
            b1rep_sb = cload("b1rep_sb", t_b1rep, [P, H])
            b2rep_sb = cload("b2rep_sb", t_b2rep, [P, H])
            Wlin_sb = cload("Wlin_sb", t_Wlin, [H, O])
            blinrep_sb = cload("blinrep_sb", t_blinrep, [P, O])
            cinv_sb = cload("cinv_sb", t_cinv, [G, 1])
            idx_sb = cload("idx_sb", t_idx, [P, 8 * TOTCH], I16)
            dstr_sb = cload("dstr_sb", t_dstr, [P, TOTCH], BF16)
            zidx_sb = cload("zidx_sb", t_zidx, [P, NBPC], I32)
            brel_sb = cload("brel_sb", t_brel, [P, NBPC])

            # folded weight tables We = [W | 0 | W@a_s | W@a_d]
            def fold(nm, tW, tWT, tas, tad):
                We = cpool.tile([H, WCOL], F32, name=nm, tag=nm)
                nc.sync.dma_start(We[:, 0:H], tW.ap())
                nc.vector.memset(We[:, 128:129], 0.0)
                WT_sb = cpool.tile([H, H], F32, name=nm + "_WT", tag=nm + "_WT")
                nc.sync.dma_start(WT_sb[:, :], tWT.ap())
                for col, tvec in ((129, tas), (130, tad)):
                    av = cpool.tile([H, 1], F32, name=f"{nm}_a{col}",
                                    tag=f"{nm}_a{col}")
                    nc.sync.dma_start(av[:, :], tvec.ap())
                    fps = pspool.tile([H, 1], F32, name=f"{nm}_f{col}", tag="tps")
                    nc.tensor.matmul(fps[:, :], lhsT=WT_sb[:, :], rhs=av[:, :],
                                     start=True, stop=True)
                    nc.scalar.activation(We[:, col:col + 1], fps[:, :], AF.Copy)
                return We

            W1e = fold("W1e", t_W1, t_W1T, t_a1s, t_a1d)
            W2e = fold("W2e", t_W2, t_W2T, t_a2s, t_a2d)

            hs1_sh = dpool.tile([SH, ROW], BF16, name="hs1_sh", tag="hs1_sh")
            hs1_f = dpool.tile([NPAD, ROW], BF16, name="hs1_f", tag="hs1_f",
                               addr_space="Shared")
            hs2_sh = dpool.tile([SH, ROW], BF16, name="hs2_sh", tag="hs2_sh")
            hs2_f = dpool.tile([NPAD, ROW], BF16, name="hs2_f", tag="hs2_f",
                               addr_space="Shared")
            pool_in = dpool.tile([G, O], F32, name="pool_in", tag="pool_in")
            pool_out = dpool.tile([G, O], F32, name="pool_out", tag="pool_out",
                                  addr_space="Shared")

            # store [h | 1 | ssrc | sdst] (f32 psum) as bf16 row, zero pad
            def store_hs(hs_ps, hs_dram, i):
                hs_sb = wpool.tile([P, ROW], BF16, name="hs_sb", tag="hs_sb")
                nc.scalar.activation(hs_sb[:, 0:WCOL], hs_ps[:, :], AF.Copy)
                nc.vector.memset(hs_sb[:, 128:129], 1.0)
                nc.vector.memset(hs_sb[:, WCOL:ROW], 0.0)
                nc.sync.dma_start(hs_dram[i * P:(i + 1) * P, :], hs_sb[:, :])

            # ---------------- stage A: hs1 for own shard ----------------
            for i in range(NBPC):
                x1 = wpool.tile([P, H], F32, name="x1", tag="x1")
                nc.gpsimd.indirect_dma_start(
                    out=x1[:, 3:128], out_offset=None, in_=t_emb.ap(),
                    in_offset=IndirectOffsetOnAxis(ap=zidx_sb[:, i:i + 1], axis=0))
                nc.sync.dma_start(x1[:, 0:3], t_pos.ap()[i * P:(i + 1) * P, :])
                xt_ps = pspool.tile([P, P], F32, name="xt_ps", tag="tps")
                nc.tensor.transpose(xt_ps[:, :], x1[:, :], ident_sb[:, :])
                x1t = wpool.tile([P, P], F32, name="x1t", tag="x1t")
                nc.scalar.activation(x1t[:, :], xt_ps[:, :], AF.Copy)
                hs_ps = pspool.tile([P, WCOL], F32, name="hs_ps", tag="hsps")
                nc.tensor.matmul(hs_ps[:, :], lhsT=x1t[:, :], rhs=W1e[:, :],
                                 start=True, stop=True)
                store_hs(hs_ps, hs1_sh, i)

            if analysis:
                nc.sync.dma_start(hs1_f[0:SH, :], hs1_sh[:, :])
            else:
                nc.gpsimd.collective_compute(
                    "AllGather", ALU.bypass, groups,
                    ins=[hs1_sh[:, :]], outs=[hs1_f[:, :]])

            # ---------------- edge phase ----------------
            def edge_phase(hs_f, hs_sh_d, post_block):
                for i in range(NBPC):
                    # block prep: s_dst row replicated across partitions
                    sd_row = spool.tile([1, P], BF16, name="sd_row",
                                        tag="sd_row")
                    nc.sync.dma_start(
                        sd_row[:, :],
                        hs_sh_d[i * P:(i + 1) * P, 130:131].transpose([1, 0]))
                    sd_ps = pspool.tile([P, P], F32, name="sd_ps", tag="tps")
                    nc.tensor.matmul(sd_ps[:, :], lhsT=ones1_sb[:, :],
                                     rhs=sd_row[:, :], start=True, stop=True)
                    sdst_rep = wpool.tile([P, P], BF16, name="sdst_rep",
                                          tag="sdst_rep")
                    nc.scalar.activation(sdst_rep[:, :], sd_ps[:, :], AF.Copy)
                    num_ps = pspool.tile([P, 129], F32, name="num_ps",
                                         tag="numps")

                    halves = []
                    if nlo[i] > 0:
                        halves.append((int(nlo[i]), int(off[i]),
                                       hs_f[0:LO, :]))
                    if nhi[i] > 0:
                        halves.append((int(nhi[i]), int(off[i]) + int(nlo[i]),
                                       hs_f[LO:NPAD, :]))

                    nch = int(nlo[i]) + int(nhi[i])
                    # emit all gathers for this block first (prefetch)
                    work = []
                    done = 0
                    for n, cbase, table in halves:
                        for s0 in range(0, n, gmax):
                            sn = min(gmax, n - s0)
                            cb = cbase + s0
                            Gt = gpool.tile([P, sn * ROW], BF16, name="Gt",
                                            tag="Gt")
                            nc.gpsimd.dma_gather(
                                out_ap=Gt.rearrange("p (c s) -> p c s", s=ROW),
                                in_ap=table,
                                idxs_ap=idx_sb[:, 8 * cb:8 * (cb + sn)],
                                num_idxs=sn * P,
                                num_idxs_reg=sn * P,
                                elem_size=ROW,
                                queue_num=next_q(),
                            )
                            work.append((Gt, sn, cb, done + s0))
                        done += n
                    for Gt, sn, cb, base in work:
                        SD = spool.tile([P, sn], F32, name="SD", tag="SD")
                        for jj in range(sn):
                            junk = mpool.tile([P, P], BF16, name="junk",
                                              tag="junk")
                            nc.vector.scalar_tensor_tensor(
                                out=junk[:, :], in0=iotab_sb[:, :],
                                scalar=dstr_sb[:, cb + jj:cb + jj + 1],
                                in1=sdst_rep[:, :],
                                op0=ALU.is_equal, op1=ALU.mult,
                                accum_out=SD[:, jj:jj + 1])
                        ssrc = Gt.rearrange("p (c s) -> p c s",
                                            s=ROW)[:, :, 129:130].squeeze(2)
                        Q = spool.tile([P, sn], F32, name="Q", tag="Q")
                        nc.vector.tensor_tensor(out=Q[:, :], in0=SD[:, :],
                                                in1=ssrc, op=ALU.add)
                        V = spool.tile([P, sn], F32, name="V", tag="V")
                        nc.vector.scalar_tensor_tensor(
                            out=V[:, :], in0=Q[:, :], scalar=NEG,
                            in1=Q[:, :], op0=ALU.mult, op1=ALU.max)
                        Pe = spool.tile([P, sn], BF16, name="Pe", tag="Pe")
                        nc.scalar.activation(Pe[:, :], V[:, :], AF.Exp)
                        for jj in range(sn):
                            MT = mpool.tile([P, P], BF16, name="MT", tag="MT")
                            nc.vector.scalar_tensor_tensor(
                                out=MT[:, :], in0=iotab_sb[:, :],
                                scalar=dstr_sb[:, cb + jj:cb + jj + 1],
                                in1=Pe[:, jj:jj + 1].to_broadcast([P, P]),
                                op0=ALU.is_equal, op1=ALU.mult)
                            if "mm" not in skip or base + jj == 0:
                                nc.tensor.matmul(
                                    num_ps[:, :], lhsT=MT[:, :],
                                    rhs=Gt[:, jj * ROW:jj * ROW + 129],
                                    start=(base + jj == 0),
                                    stop=(base + jj == nch - 1)
                                    if "mm" not in skip else True)
                    post_block(i, num_ps)

            # common post-block epilogue: x = elu(num/den + b)
            def finish_x(num_ps, brep_sb):
                den = spool.tile([P, 1], F32, name="den", tag="den")
                nc.vector.tensor_scalar(out=den[:, :], in0=num_ps[:, 128:129],
                                        scalar1=1e-30, scalar2=None, op0=ALU.max)
                rec = spool.tile([P, 1], F32, name="rec", tag="rec")
                nc.vector.reciprocal(rec[:, :], den[:, :])
                xp = wpool.tile([P, H], F32, name="xp", tag="xp")
                nc.vector.scalar_tensor_tensor(
                    out=xp[:, :], in0=num_ps[:, 0:128], scalar=rec[:, :],
                    in1=brep_sb[:, :], op0=ALU.mult, op1=ALU.add)
                xm = wpool.tile([P, H], F32, name="xm", tag="xm")
                nc.vector.tensor_scalar(out=xm[:, :], in0=xp[:, :], scalar1=0.0,
                                        scalar2=None, op0=ALU.min)
                xe = wpool.tile([P, H], F32, name="xe", tag="xe")
                nc.scalar.activation(xe[:, :], xm[:, :], AF.Exp)
                xr = wpool.tile([P, H], F32, name="xr", tag="xr")
                nc.vector.tensor_scalar(out=xr[:, :], in0=xp[:, :], scalar1=0.0,
                                        scalar2=None, op0=ALU.max)
                x2 = wpool.tile([P, H], F32, name="x2", tag="x2")
                nc.vector.scalar_tensor_tensor(
                    out=x2[:, :], in0=xe[:, :], scalar=-1.0, in1=xr[:, :],
                    op0=ALU.add, op1=ALU.add)
                return x2

            # layer-1 post: x2 -> hs2 shard rows
            def post1(i, num_ps):
                x2 = finish_x(num_ps, b1rep_sb)
                xt_ps = pspool.tile([P, P], F32, name="x2t_ps", tag="tps")
                nc.tensor.transpose(xt_ps[:, :], x2[:, :], ident_sb[:, :])
                x2t = wpool.tile([P, P], F32, name="x2t", tag="x2t")
                nc.scalar.activation(x2t[:, :], xt_ps[:, :], AF.Copy)
                hs_ps = pspool.tile([P, WCOL], F32, name="hs2_ps", tag="hsps")
                nc.tensor.matmul(hs_ps[:, :], lhsT=x2t[:, :], rhs=W2e[:, :],
                                 start=True, stop=True)
                store_hs(hs_ps, hs2_sh, i)

            edge_phase(hs1_f, hs1_sh, post1)

            if analysis:
                nc.sync.dma_start(hs2_f[0:SH, :], hs2_sh[:, :])
            else:
                nc.gpsimd.collective_compute(
                    "AllGather", ALU.bypass, groups,
                    ins=[hs2_sh[:, :]], outs=[hs2_f[:, :]])

            # layer-2 post: y = x3 @ Wlin + blin; pool matmul accumulate
            pool_ps = apool.tile([G, O], F32, name="pool_ps", tag="poolps")

            def post2(i, num_ps):
                x3 = finish_x(num_ps, b2rep_sb)
                xt_ps = pspool.tile([P, P], F32, name="x3t_ps", tag="tps")
                nc.tensor.transpose(xt_ps[:, :], x3[:, :], ident_sb[:, :])
                x3t = wpool.tile([P, P], F32, name="x3t", tag="x2t")
                nc.scalar.activation(x3t[:, :], xt_ps[:, :], AF.Copy)
                y_ps = pspool.tile([P, O], F32, name="y_ps", tag="hsps")
                nc.tensor.matmul(y_ps[:, :], lhsT=x3t[:, :], rhs=Wlin_sb[:, :],
                                 start=True, stop=True)
                y_sb = wpool.tile([P, O], F32, name="y_sb", tag="y_sb")
                nc.vector.tensor_tensor(out=y_sb[:, :], in0=y_ps[:, :],
                                        in1=blinrep_sb[:, :], op=ALU.add)
                Mg = wpool.tile([P, G], F32, name="Mg", tag="Mg")
                nc.vector.tensor_scalar(out=Mg[:, :], in0=iota_sb[:, 0:G],
                                        scalar1=brel_sb[:, i:i + 1],
                                        scalar2=None, op0=ALU.is_equal)
                nc.tensor.matmul(pool_ps[:, :], lhsT=Mg[:, :], rhs=y_sb[:, :],
                                 start=(i == 0), stop=(i == NBPC - 1))

            edge_phase(hs2_f, hs2_sh, post2)

            # ---------------- final reduce ----------------
            pool_sb = spool.tile([G, O], F32, name="pool_sb", tag="pool_sb")
            nc.scalar.activation(pool_sb[:, :], pool_ps[:, :], AF.Copy)
            nc.sync.dma_start(pool_in[:, :], pool_sb[:, :])
            if analysis:
                nc.sync.dma_start(pool_out[:, :], pool_in[:, :])
            else:
                nc.gpsimd.collective_compute(
                    "AllReduce", ALU.add, groups,
                    ins=[pool_in[:, :]], outs=[pool_out[:, :]])
            red_sb = spool.tile([G, O], F32, name="red_sb", tag="red_sb")
            nc.sync.dma_start(red_sb[:, :], pool_out[:, :])
            fin_sb = spool.tile([G, O], F32, name="fin_sb", tag="fin_sb")
            nc.vector.tensor_scalar(out=fin_sb[:, :], in0=red_sb[:, :],
                                    scalar1=cinv_sb[:, :], scalar2=None,
                                    op0=ALU.mult)
            nc.sync.dma_start(t_out.ap(), fin_sb[:, :])

    nc.compile()
    nc.m = get_hw_module(nc.m)
    return nc


_CACHE = {}


def _get_nc(meta):
    key = (meta["TOTCH"], meta["nlo"], meta["nhi"])
    if key not in _CACHE:
        _CACHE[key] = _build(meta)
    return _CACHE[key]


def run(inputs, trace=False, **kw):
    meta, percore, consts = _prep(inputs)
    nc = _get_nc(meta)
    in_maps = []
    for c in range(NCORES):
        m = dict(consts)
        m["idx_w"] = np.ascontiguousarray(percore["idx_w"][c])
        m["dstrel"] = np.ascontiguousarray(percore["dstrel"][c])
        m["zidx"] = np.ascontiguousarray(percore["zidx"][c])
        m["batchrel"] = np.ascontiguousarray(percore["batchrel"][c])
        m["pos_shard"] = np.ascontiguousarray(percore["pos_shard"][c])
        in_maps.append(m)
    res = bass_utils.run_bass_kernel_spmd(
        nc, in_maps, core_ids=list(range(NCORES)), trace=trace, **kw)
    return res


def kernel(**inputs):
    res = run(inputs, trace=False)
    return res.results[0]["out"]



# revision 2
# speedup vs baseline: 1.1278x; 1.1278x over previous
"""Trainium2 Bass kernel for nn_EquivariantGNN_GAT (2-layer GAT + linear + mean pool).

Strategy (8 NeuronCores, SPMD single program):
  - Nodes padded to 50176 = 392 blocks of 128; each core owns 49 dst-blocks
    (6272 nodes) and all edges incident (by dst) on them, host-sorted by dst.
  - Per layer, each core computes hs = x @ [W | 0 | W@a_src | W@a_dst] for its
    node shard in f32, stores the per-node row [h(128) | 1 | s_src | s_dst]
    cast to bf16 (512B rows), AllGathers the full [50176, 256] bf16 table
    into HBM, then processes its edges in chunks of 128 via dma_gather of
    hs[src] rows. int16 gather indices are handled by splitting each block's
    edges into src<32768 ("lo") and src>=32768 ("hi") halves gathered from
    offset table views; gathers are capped at 8 chunks (1024 descriptors)
    to fit the SWDGE ring.
  - Per chunk: one-hot dst matrix scaled by exp(leaky_relu(s_src + s_dst))
    built on DVE (scalar_tensor_tensor with fused accum for the s_dst
    expansion), then a single bf16 matmul accumulates numerator + softmax
    denominator ([h | 1] columns) into f32 PSUM per dst block.
  - Softmax max-subtraction is skipped (mathematically equivalent here).
  - Final: y = x3 @ Wlin + blin per block, per-graph mean pool via one-hot
    matmul accumulated in PSUM, AllReduce over cores, scale by 1/counts.

kernel(**inputs) takes the FULL problem inputs and returns the [64, 32] output.
"""
import sys

sys.path.insert(0, "/opt/trn_rl_repo")

import ml_dtypes
import numpy as np

import concourse.bass as bass
import concourse.bacc as bacc
import concourse.mybir as mybir
import concourse.tile as tile
import concourse.bass_utils as bass_utils
from concourse.bass import IndirectOffsetOnAxis
from concourse.bass_interp import get_hw_module

N = 50000
E = 1600000
H = 128
O = 32
T = 100
G = 64
P = 128
NCORES = 8
NBPC = 49              # dst blocks per core
NB = NBPC * NCORES     # 392 blocks -> 50176 padded nodes
NPAD = NB * P
SH = NBPC * P          # 6272 nodes per core
ROW = 256              # bf16 elems per hs row: [h(128) | 1 | ssrc | sdst | 0pad]
WCOL = 131             # computed columns: [W(128) | 0 | W@a_s | W@a_d]
LO = 32768             # int16 index limit; src >= LO gathered from offset view
GMAX = 8               # chunks per dma_gather (1024 descs fit the SWDGE ring)
NEG = 0.2

F32 = mybir.dt.float32
BF16 = mybir.dt.bfloat16
I32 = mybir.dt.int32
I16 = mybir.dt.int16
ALU = mybir.AluOpType
AF = mybir.ActivationFunctionType
NPBF = ml_dtypes.bfloat16


# ---------------------------------------------------------------- host prep
def _wrap16(flat):
    """dma_gather index layout: idx k -> [k%16, k//16], replicated x8."""
    n = flat.shape[0]
    assert n % 16 == 0
    w = flat.reshape(n // 16, 16).T          # [16, n//16]
    return np.tile(w, (8, 1))                 # [128, n//16]


def _prep(inputs):
    pos = np.ascontiguousarray(np.asarray(inputs["pos"], np.float32))
    z = np.asarray(inputs["z"]).astype(np.int64)
    ei = np.asarray(inputs["edge_index"]).astype(np.int64)
    batch = np.asarray(inputs["batch"]).astype(np.int64)

    loop = np.arange(N, dtype=np.int64)
    src = np.concatenate([ei[0], loop])
    dst = np.concatenate([ei[1], loop])
    order = np.argsort(dst, kind="stable")
    src_s = src[order].astype(np.int32)
    dst_s = dst[order].astype(np.int32)

    bnd = np.searchsorted(dst_s, np.arange(NB + 1) * P).astype(np.int64)

    block_lo = {}
    block_hi = {}
    for b in range(NB):
        e0, e1 = int(bnd[b]), int(bnd[b + 1])
        s = src_s[e0:e1]
        d = dst_s[e0:e1] - b * P
        m = s < LO
        block_lo[b] = (s[m], d[m])
        block_hi[b] = (s[~m] - LO, d[~m])

    nlo = np.zeros(NBPC, np.int64)
    nhi = np.zeros(NBPC, np.int64)
    for i in range(NBPC):
        for c in range(NCORES):
            b = NBPC * c + i
            nlo[i] = max(nlo[i], (len(block_lo[b][0]) + P - 1) // P)
            nhi[i] = max(nhi[i], (len(block_hi[b][0]) + P - 1) // P)
        if nlo[i] + nhi[i] == 0:
            nhi[i] = 1
    CH = nlo + nhi
    off = np.zeros(NBPC + 1, np.int64)
    off[1:] = np.cumsum(CH)
    TOTCH = int(off[-1])

    idx_w = np.zeros((NCORES, P, 8 * TOTCH), np.int16)
    dstrel = np.full((NCORES, P, TOTCH), -1.0, NPBF)
    zidx = np.zeros((NCORES, P, NBPC), np.int32)
    batchrel = np.full((NCORES, P, NBPC), -1.0, np.float32)
    pos_shard = np.zeros((NCORES, SH, 3), np.float32)

    for c in range(NCORES):
        for i in range(NBPC):
            b = NBPC * c + i
            for (s, d), n_ch, cbase in (
                (block_lo[b], int(nlo[i]), int(off[i])),
                (block_hi[b], int(nhi[i]), int(off[i]) + int(nlo[i])),
            ):
                if n_ch == 0:
                    continue
                ne = len(s)
                flat = np.zeros(n_ch * P, np.int16)
                flat[:ne] = s.astype(np.int16)
                dr = np.full(n_ch * P, -1.0, np.float32)
                dr[:ne] = d.astype(np.float32)
                idx_w[c, :, 8 * cbase:8 * (cbase + n_ch)] = _wrap16(flat)
                dstrel[c, :, cbase:cbase + n_ch] = \
                    dr.reshape(n_ch, P).T.astype(NPBF)
        n0 = SH * c
        nodes = np.arange(n0, n0 + SH)
        valid = nodes < N
        zc = np.zeros(SH, np.int32)
        zc[valid] = z[nodes[valid]].astype(np.int32)
        zidx[c] = zc.reshape(NBPC, P).T
        bc = np.full(SH, -1.0, np.float32)
        bc[valid] = batch[nodes[valid]].astype(np.float32)
        batchrel[c] = bc.reshape(NBPC, P).T
        pos_shard[c][valid] = pos[nodes[valid]]

    counts = np.bincount(batch, minlength=G).astype(np.float32)
    cinv = (1.0 / np.maximum(counts, 1.0)).astype(np.float32).reshape(G, 1)

    iota_f = np.tile(np.arange(P, dtype=np.float32), (P, 1))
    consts = dict(
        iota=np.ascontiguousarray(iota_f),
        iotab=np.ascontiguousarray(iota_f.astype(NPBF)),
        ident=np.eye(P, dtype=np.float32),
        ones1=np.ones((1, P), NPBF),
        W1=np.ascontiguousarray(np.asarray(inputs["W1"], np.float32)),
        W1T=np.ascontiguousarray(np.asarray(inputs["W1"], np.float32).T),
        a1s=np.asarray(inputs["a1_src"], np.float32).reshape(H, 1),
        a1d=np.asarray(inputs["a1_dst"], np.float32).reshape(H, 1),
        b1rep=np.ascontiguousarray(
            np.tile(np.asarray(inputs["b1"], np.float32), (P, 1))),
        W2=np.ascontiguousarray(np.asarray(inputs["W2"], np.float32)),
        W2T=np.ascontiguousarray(np.asarray(inputs["W2"], np.float32).T),
        a2s=np.asarray(inputs["a2_src"], np.float32).reshape(H, 1),
        a2d=np.asarray(inputs["a2_dst"], np.float32).reshape(H, 1),
        b2rep=np.ascontiguousarray(
            np.tile(np.asarray(inputs["b2"], np.float32), (P, 1))),
        Wlin=np.ascontiguousarray(np.asarray(inputs["Wlin"], np.float32)),
        blinrep=np.ascontiguousarray(
            np.tile(np.asarray(inputs["blin"], np.float32), (P, 1))),
        emb=np.ascontiguousarray(np.asarray(inputs["emb"], np.float32)),
        cinv=cinv,
    )

    meta = dict(nlo=tuple(int(x) for x in nlo),
                nhi=tuple(int(x) for x in nhi),
                off=tuple(int(x) for x in off),
                TOTCH=TOTCH)
    percore = dict(idx_w=idx_w, dstrel=dstrel, zidx=zidx,
                   batchrel=batchrel, pos_shard=pos_shard)
    return meta, percore, consts


# ---------------------------------------------------------------- program
def _build(meta, analysis=False, gmax=GMAX, nqueues=1, skip=(),
           scratch=16384, gbufs=3, mtact=False, bufboost=0):
    nlo = meta["nlo"]
    nhi = meta["nhi"]
    off = meta["off"]
    TOTCH = meta["TOTCH"]
    qstate = {"q": 0}

    def next_q():
        q = qstate["q"]
        qstate["q"] = (q + 1) % nqueues
        return q

    nc = bacc.Bacc("TRN2", target_bir_lowering=False, debug=False,
                   enable_asserts=False,
                   num_devices=1 if analysis else NCORES,
                   num_swdge_queues=nqueues,
                   dynamic_dma_scratch_size=scratch)

    t_idx = nc.dram_tensor("idx_w", [P, 8 * TOTCH], I16, kind="ExternalInput")
    t_dstr = nc.dram_tensor("dstrel", [P, TOTCH], BF16, kind="ExternalInput")
    t_zidx = nc.dram_tensor("zidx", [P, NBPC], I32, kind="ExternalInput")
    t_brel = nc.dram_tensor("batchrel", [P, NBPC], F32, kind="ExternalInput")
    t_pos = nc.dram_tensor("pos_shard", [SH, 3], F32, kind="ExternalInput")
    t_emb = nc.dram_tensor("emb", [T, 125], F32, kind="ExternalInput")
    t_iota = nc.dram_tensor("iota", [P, P], F32, kind="ExternalInput")
    t_iotab = nc.dram_tensor("iotab", [P, P], BF16, kind="ExternalInput")
    t_ident = nc.dram_tensor("ident", [P, P], F32, kind="ExternalInput")
    t_ones1 = nc.dram_tensor("ones1", [1, P], BF16, kind="ExternalInput")
    t_W1 = nc.dram_tensor("W1", [H, H], F32, kind="ExternalInput")
    t_W1T = nc.dram_tensor("W1T", [H, H], F32, kind="ExternalInput")
    t_a1s = nc.dram_tensor("a1s", [H, 1], F32, kind="ExternalInput")
    t_a1d = nc.dram_tensor("a1d", [H, 1], F32, kind="ExternalInput")
    t_b1rep = nc.dram_tensor("b1rep", [P, H], F32, kind="ExternalInput")
    t_W2 = nc.dram_tensor("W2", [H, H], F32, kind="ExternalInput")
    t_W2T = nc.dram_tensor("W2T", [H, H], F32, kind="ExternalInput")
    t_a2s = nc.dram_tensor("a2s", [H, 1], F32, kind="ExternalInput")
    t_a2d = nc.dram_tensor("a2d", [H, 1], F32, kind="ExternalInput")
    t_b2rep = nc.dram_tensor("b2rep", [P, H], F32, kind="ExternalInput")
    t_Wlin = nc.dram_tensor("Wlin", [H, O], F32, kind="ExternalInput")
    t_blinrep = nc.dram_tensor("blinrep", [P, O], F32, kind="ExternalInput")
    t_cinv = nc.dram_tensor("cinv", [G, 1], F32, kind="ExternalInput")
    t_out = nc.dram_tensor("out", [G, O], F32, kind="ExternalOutput")

    groups = [list(range(NCORES))]

    with tile.TileContext(nc) as tc:
        with (
            tc.tile_pool(name="const", bufs=1) as cpool,
            tc.tile_pool(name="work", bufs=3) as wpool,
            tc.tile_pool(name="gat", bufs=gbufs) as gpool,
            tc.tile_pool(name="mt", bufs=14 + bufboost) as mpool,
            tc.tile_pool(name="gp", bufs=6 + bufboost) as gppool,
            tc.tile_pool(name="sml", bufs=4 + bufboost) as spool,
            tc.tile_pool(name="ps", bufs=2, space="PSUM") as pspool,
            tc.tile_pool(name="acc", bufs=1, space="PSUM") as apool,
            tc.tile_pool(name="dram", bufs=1, space="DRAM") as dpool,
        ):
            def cload(nm, t, shape, dtype=F32):
                tl = cpool.tile(shape, dtype, name=nm, tag=nm)
                nc.sync.dma_start(tl[:, :], t.ap())
                return tl

            iota_sb = cload("iota_sb", t_iota, [P, P])
            iotab_sb = cload("iotab_sb", t_iotab, [P, P], BF16)
            ident_sb = cload("ident_sb", t_ident, [P, P])
            ones1_sb = cload("ones1_sb", t_ones1, [1, P], BF16)
            eps_sb = cpool.tile([P, 1], F32, name="eps_sb", tag="eps_sb")
            nc.vector.memset(eps_sb[:, :], 1e-38)
            b1rep_sb = cload("b1rep_sb", t_b1rep, [P, H])
            b2rep_sb = cload("b2rep_sb", t_b2rep, [P, H])
            Wlin_sb = cload("Wlin_sb", t_Wlin, [H, O])
            blinrep_sb = cload("blinrep_sb", t_blinrep, [P, O])
            cinv_sb = cload("cinv_sb", t_cinv, [G, 1])
            idx_sb = cload("idx_sb", t_idx, [P, 8 * TOTCH], I16)
            dstr_sb = cload("dstr_sb", t_dstr, [P, TOTCH], BF16)
            zidx_sb = cload("zidx_sb", t_zidx, [P, NBPC], I32)
            brel_sb = cload("brel_sb", t_brel, [P, NBPC])

            # folded weight tables We = [W | 0 | W@a_s | W@a_d]
            def fold(nm, tW, tWT, tas, tad):
                We = cpool.tile([H, WCOL], F32, name=nm, tag=nm)
                nc.sync.dma_start(We[:, 0:H], tW.ap())
                nc.vector.memset(We[:, 128:129], 0.0)
                WT_sb = cpool.tile([H, H], F32, name=nm + "_WT", tag=nm + "_WT")
                nc.sync.dma_start(WT_sb[:, :], tWT.ap())
                for col, tvec in ((129, tas), (130, tad)):
                    av = cpool.tile([H, 1], F32, name=f"{nm}_a{col}",
                                    tag=f"{nm}_a{col}")
                    nc.sync.dma_start(av[:, :], tvec.ap())
                    fps = pspool.tile([H, 1], F32, name=f"{nm}_f{col}", tag="tps")
                    nc.tensor.matmul(fps[:, :], lhsT=WT_sb[:, :], rhs=av[:, :],
                                     start=True, stop=True)
                    nc.scalar.activation(We[:, col:col + 1], fps[:, :], AF.Copy)
                return We

            W1e = fold("W1e", t_W1, t_W1T, t_a1s, t_a1d)
            W2e = fold("W2e", t_W2, t_W2T, t_a2s, t_a2d)

            hs1_sh = dpool.tile([SH, ROW], BF16, name="hs1_sh", tag="hs1_sh")
            hs1_f = dpool.tile([NPAD, ROW], BF16, name="hs1_f", tag="hs1_f",
                               addr_space="Shared")
            hs2_sh = dpool.tile([SH, ROW], BF16, name="hs2_sh", tag="hs2_sh")
            hs2_f = dpool.tile([NPAD, ROW], BF16, name="hs2_f", tag="hs2_f",
                               addr_space="Shared")
            pool_in = dpool.tile([G, O], F32, name="pool_in", tag="pool_in")
            pool_out = dpool.tile([G, O], F32, name="pool_out", tag="pool_out",
                                  addr_space="Shared")

            # store [h | 1 | ssrc | sdst] (f32 psum) as bf16 row, zero pad
            def store_hs(hs_ps, hs_dram, i):
                hs_sb = wpool.tile([P, ROW], BF16, name="hs_sb", tag="hs_sb")
                nc.scalar.activation(hs_sb[:, 0:WCOL], hs_ps[:, :], AF.Copy)
                nc.vector.memset(hs_sb[:, 128:129], 1.0)
                nc.vector.memset(hs_sb[:, WCOL:ROW], 0.0)
                nc.sync.dma_start(hs_dram[i * P:(i + 1) * P, :], hs_sb[:, :])

            # ---------------- stage A: hs1 for own shard ----------------
            for i in range(NBPC):
                x1 = wpool.tile([P, H], F32, name="x1", tag="x1")
                nc.gpsimd.indirect_dma_start(
                    out=x1[:, 3:128], out_offset=None, in_=t_emb.ap(),
                    in_offset=IndirectOffsetOnAxis(ap=zidx_sb[:, i:i + 1], axis=0))
                nc.sync.dma_start(x1[:, 0:3], t_pos.ap()[i * P:(i + 1) * P, :])
                xt_ps = pspool.tile([P, P], F32, name="xt_ps", tag="tps")
                nc.tensor.transpose(xt_ps[:, :], x1[:, :], ident_sb[:, :])
                x1t = wpool.tile([P, P], F32, name="x1t", tag="x1t")
                nc.scalar.activation(x1t[:, :], xt_ps[:, :], AF.Copy)
                hs_ps = pspool.tile([P, WCOL], F32, name="hs_ps", tag="hsps")
                nc.tensor.matmul(hs_ps[:, :], lhsT=x1t[:, :], rhs=W1e[:, :],
                                 start=True, stop=True)
                store_hs(hs_ps, hs1_sh, i)

            if analysis:
                nc.sync.dma_start(hs1_f[0:SH, :], hs1_sh[:, :])
            else:
                nc.gpsimd.collective_compute(
                    "AllGather", ALU.bypass, groups,
                    ins=[hs1_sh[:, :]], outs=[hs1_f[:, :]])

            # ---------------- edge phase ----------------
            def edge_phase(hs_f, hs_sh_d, post_block):
                for i in range(NBPC):
                    # block prep: s_dst row replicated across partitions
                    sd_row = spool.tile([1, P], BF16, name="sd_row",
                                        tag="sd_row")
                    nc.sync.dma_start(
                        sd_row[:, :],
                        hs_sh_d[i * P:(i + 1) * P, 130:131].transpose([1, 0]))
                    sd_ps = pspool.tile([P, P], F32, name="sd_ps", tag="tps")
                    nc.tensor.matmul(sd_ps[:, :], lhsT=ones1_sb[:, :],
                                     rhs=sd_row[:, :], start=True, stop=True)
                    sdst_rep = wpool.tile([P, P], BF16, name="sdst_rep",
                                          tag="sdst_rep")
                    nc.scalar.activation(sdst_rep[:, :], sd_ps[:, :], AF.Copy)
                    num_ps = pspool.tile([P, 129], F32, name="num_ps",
                                         tag="numps")

                    halves = []
                    if nlo[i] > 0:
                        halves.append((int(nlo[i]), int(off[i]),
                                       hs_f[0:LO, :]))
                    if nhi[i] > 0:
                        halves.append((int(nhi[i]), int(off[i]) + int(nlo[i]),
                                       hs_f[LO:NPAD, :]))

                    nch = int(nlo[i]) + int(nhi[i])
                    # emit all gathers for this block first (prefetch)
                    work = []
                    done = 0
                    for n, cbase, table in halves:
                        for s0 in range(0, n, gmax):
                            sn = min(gmax, n - s0)
                            cb = cbase + s0
                            Gt = gpool.tile([P, sn * ROW], BF16, name="Gt",
                                            tag="Gt")
                            nc.gpsimd.dma_gather(
                                out_ap=Gt.rearrange("p (c s) -> p c s", s=ROW),
                                in_ap=table,
                                idxs_ap=idx_sb[:, 8 * cb:8 * (cb + sn)],
                                num_idxs=sn * P,
                                num_idxs_reg=sn * P,
                                elem_size=ROW,
                                queue_num=next_q(),
                            )
                            work.append((Gt, sn, cb, done + s0))
                        done += n
                    for Gt, sn, cb, base in work:
                        SD = spool.tile([P, sn], F32, name="SD", tag="SD")
                        for jj in range(sn):
                            junk = mpool.tile([P, P], BF16, name="junk",
                                              tag="junk")
                            nc.vector.scalar_tensor_tensor(
                                out=junk[:, :], in0=iotab_sb[:, :],
                                scalar=dstr_sb[:, cb + jj:cb + jj + 1],
                                in1=sdst_rep[:, :],
                                op0=ALU.is_equal, op1=ALU.mult,
                                accum_out=SD[:, jj:jj + 1])
                        ssrc = Gt.rearrange("p (c s) -> p c s",
                                            s=ROW)[:, :, 129:130].squeeze(2)
                        Q = spool.tile([P, sn], F32, name="Q", tag="Q")
                        nc.vector.tensor_tensor(out=Q[:, :], in0=SD[:, :],
                                                in1=ssrc, op=ALU.add)
                        V = spool.tile([P, sn], F32, name="V", tag="V")
                        nc.vector.scalar_tensor_tensor(
                            out=V[:, :], in0=Q[:, :], scalar=NEG,
                            in1=Q[:, :], op0=ALU.mult, op1=ALU.max)
                        Pe = spool.tile([P, sn], BF16, name="Pe", tag="Pe")
                        nc.scalar.activation(Pe[:, :], V[:, :], AF.Exp)
                        for jj in range(sn):
                            MT = mpool.tile([P, P], BF16, name="MT", tag="MT")
                            nc.vector.scalar_tensor_tensor(
                                out=MT[:, :], in0=iotab_sb[:, :],
                                scalar=dstr_sb[:, cb + jj:cb + jj + 1],
                                in1=Pe[:, jj:jj + 1].to_broadcast([P, P]),
                                op0=ALU.is_equal, op1=ALU.mult)
                            if "mm" not in skip or base + jj == 0:
                                nc.tensor.matmul(
                                    num_ps[:, :], lhsT=MT[:, :],
                                    rhs=Gt[:, jj * ROW:jj * ROW + 129],
                                    start=(base + jj == 0),
                                    stop=(base + jj == nch - 1)
                                    if "mm" not in skip else True)
                    post_block(i, num_ps)

            # common post-block epilogue: x = elu(num/den + b)
            def finish_x(num_ps, brep_sb):
                den = spool.tile([P, 1], F32, name="den", tag="den")
                nc.vector.tensor_scalar(out=den[:, :], in0=num_ps[:, 128:129],
                                        scalar1=1e-30, scalar2=None, op0=ALU.max)
                rec = spool.tile([P, 1], F32, name="rec", tag="rec")
                nc.vector.reciprocal(rec[:, :], den[:, :])
                xp = wpool.tile([P, H], F32, name="xp", tag="xp")
                nc.vector.scalar_tensor_tensor(
                    out=xp[:, :], in0=num_ps[:, 0:128], scalar=rec[:, :],
                    in1=brep_sb[:, :], op0=ALU.mult, op1=ALU.add)
                xm = wpool.tile([P, H], F32, name="xm", tag="xm")
                nc.vector.tensor_scalar(out=xm[:, :], in0=xp[:, :], scalar1=0.0,
                                        scalar2=None, op0=ALU.min)
                xe = wpool.tile([P, H], F32, name="xe", tag="xe")
                nc.scalar.activation(xe[:, :], xm[:, :], AF.Exp)
                xr = wpool.tile([P, H], F32, name="xr", tag="xr")
                nc.vector.tensor_scalar(out=xr[:, :], in0=xp[:, :], scalar1=0.0,
                                        scalar2=None, op0=ALU.max)
                x2 = wpool.tile([P, H], F32, name="x2", tag="x2")
                nc.vector.scalar_tensor_tensor(
                    out=x2[:, :], in0=xe[:, :], scalar=-1.0, in1=xr[:, :],
                    op0=ALU.add, op1=ALU.add)
                return x2

            # layer-1 post: x2 -> hs2 shard rows
            def post1(i, num_ps):
                x2 = finish_x(num_ps, b1rep_sb)
                xt_ps = pspool.tile([P, P], F32, name="x2t_ps", tag="tps")
                nc.tensor.transpose(xt_ps[:, :], x2[:, :], ident_sb[:, :])
                x2t = wpool.tile([P, P], F32, name="x2t", tag="x2t")
                nc.scalar.activation(x2t[:, :], xt_ps[:, :], AF.Copy)
                hs_ps = pspool.tile([P, WCOL], F32, name="hs2_ps", tag="hsps")
                nc.tensor.matmul(hs_ps[:, :], lhsT=x2t[:, :], rhs=W2e[:, :],
                                 start=True, stop=True)
                store_hs(hs_ps, hs2_sh, i)

            edge_phase(hs1_f, hs1_sh, post1)

            if analysis:
                nc.sync.dma_start(hs2_f[0:SH, :], hs2_sh[:, :])
            else:
                nc.gpsimd.collective_compute(
                    "AllGather", ALU.bypass, groups,
                    ins=[hs2_sh[:, :]], outs=[hs2_f[:, :]])

            # layer-2 post: y = x3 @ Wlin + blin; pool matmul accumulate
            pool_ps = apool.tile([G, O], F32, name="pool_ps", tag="poolps")

            def post2(i, num_ps):
                x3 = finish_x(num_ps, b2rep_sb)
                xt_ps = pspool.tile([P, P], F32, name="x3t_ps", tag="tps")
                nc.tensor.transpose(xt_ps[:, :], x3[:, :], ident_sb[:, :])
                x3t = wpool.tile([P, P], F32, name="x3t", tag="x2t")
                nc.scalar.activation(x3t[:, :], xt_ps[:, :], AF.Copy)
                y_ps = pspool.tile([P, O], F32, name="y_ps", tag="hsps")
                nc.tensor.matmul(y_ps[:, :], lhsT=x3t[:, :], rhs=Wlin_sb[:, :],
                                 start=True, stop=True)
                y_sb = wpool.tile([P, O], F32, name="y_sb", tag="y_sb")
                nc.vector.tensor_tensor(out=y_sb[:, :], in0=y_ps[:, :],
                                        in1=blinrep_sb[:, :], op=ALU.add)
                Mg = wpool.tile([P, G], F32, name="Mg", tag="Mg")
                nc.vector.tensor_scalar(out=Mg[:, :], in0=iota_sb[:, 0:G],
                                        scalar1=brel_sb[:, i:i + 1],
                                        scalar2=None, op0=ALU.is_equal)
                nc.tensor.matmul(pool_ps[:, :], lhsT=Mg[:, :], rhs=y_sb[:, :],
                                 start=(i == 0), stop=(i == NBPC - 1))

            edge_phase(hs2_f, hs2_sh, post2)

            # ---------------- final reduce ----------------
            pool_sb = spool.tile([G, O], F32, name="pool_sb", tag="pool_sb")
            nc.scalar.activation(pool_sb[:, :], pool_ps[:, :], AF.Copy)
            nc.sync.dma_start(pool_in[:, :], pool_sb[:, :])
            if analysis:
                nc.sync.dma_start(pool_out[:, :], pool_in[:, :])
            else:
                nc.gpsimd.collective_compute(
                    "AllReduce", ALU.add, groups,
                    ins=[pool_in[:, :]], outs=[pool_out[:, :]])
            red_sb = spool.tile([G, O], F32, name="red_sb", tag="red_sb")
            nc.sync.dma_start(red_sb[:, :], pool_out[:, :])
            fin_sb = spool.tile([G, O], F32, name="fin_sb", tag="fin_sb")
            nc.vector.tensor_scalar(out=fin_sb[:, :], in0=red_sb[:, :],
                                    scalar1=cinv_sb[:, :], scalar2=None,
                                    op0=ALU.mult)
            nc.sync.dma_start(t_out.ap(), fin_sb[:, :])

    nc.compile()
    nc.m = get_hw_module(nc.m)
    return nc


_CACHE = {}


def _get_nc(meta):
    key = (meta["TOTCH"], meta["nlo"], meta["nhi"])
    if key not in _CACHE:
        _CACHE[key] = _build(meta, nqueues=4)
    return _CACHE[key]


def run(inputs, trace=False, **kw):
    meta, percore, consts = _prep(inputs)
    nc = _get_nc(meta)
    in_maps = []
    for c in range(NCORES):
        m = dict(consts)
        m["idx_w"] = np.ascontiguousarray(percore["idx_w"][c])
        m["dstrel"] = np.ascontiguousarray(percore["dstrel"][c])
        m["zidx"] = np.ascontiguousarray(percore["zidx"][c])
        m["batchrel"] = np.ascontiguousarray(percore["batchrel"][c])
        m["pos_shard"] = np.ascontiguousarray(percore["pos_shard"][c])
        in_maps.append(m)
    res = bass_utils.run_bass_kernel_spmd(
        nc, in_maps, core_ids=list(range(NCORES)), trace=trace, **kw)
    return res


def kernel(**inputs):
    res = run(inputs, trace=False)
    return res.results[0]["out"]



# revision 9
# speedup vs baseline: 1.7641x; 1.5642x over previous
"""Trainium2 Bass kernel for nn_EquivariantGNN_GAT (2-layer GAT + linear + mean pool).

v2 strategy (8 NeuronCores, SPMD single program):
  - Nodes padded to 50176 = 392 blocks of 128; each core owns 49 dst-blocks
    and all edges incident (by dst) on them, host-sorted by dst; per block the
    edges are split into src<32768 (lo) / >=32768 (hi) halves and chunked by
    128 (int16 SWDGE gather indices).
  - Per layer each core computes hs = x @ [W | W@a_src | W@a_dst] for its
    shard; stores per-node 256B fp8 rows [h_fp8(128) | 1.0 | ssrc_bf16(2B) |
    pad]; AllGathers the [50176, 256] fp8 table to HBM; s_dst columns stay in
    SBUF (per own block).
  - Edge phase per block: dma_gather of fp8 rows (4 SWDGE queues round-robin,
    8 gather bufs for queue overlap); s_dst per edge via per-chunk PE matvec
    with a static fp8 one-hot-transpose streamed from HBM; scores Q/V/exp
    batched per gather group on DVE+Act; MT (alpha-scaled one-hot) built per
    block in TWO batched DVE tensor_tensor ops in a c-innermost layout;
    aggregation matmul lhsT=MT(bf16), rhs=gathered fp8 rows [h|1] accumulates
    numerator+denominator per dst block in PSUM.
  - Final: y = x3 @ Wlin + blin, per-graph mean pool via one-hot matmul,
    AllReduce, scale by 1/counts.

kernel(**inputs) takes the FULL problem inputs and returns the [64, 32] output.
"""
import sys

sys.path.insert(0, "/opt/trn_rl_repo")

import ml_dtypes
import numpy as np

import concourse.bass as bass
import concourse.bacc as bacc
import concourse.mybir as mybir
import concourse.tile as tile
import concourse.bass_utils as bass_utils
from concourse.bass import IndirectOffsetOnAxis
from concourse.bass_interp import get_hw_module

N = 50000
E = 1600000
H = 128
O = 32
T = 100
G = 64
P = 128
NCORES = 8
NBPC = 49              # dst blocks per core
NB = NBPC * NCORES     # 392 blocks -> 50176 padded nodes
NPAD = NB * P
SH = NBPC * P          # 6272 nodes per core
ROW = 256              # fp8 bytes per hs row: [h(128) | 1 | ssrc(2B) | pad]
WCOL = 130             # hs matmul cols: [W(128) | W@a_s | W@a_d]
LO = 32768             # int16 idx limit; src >= LO gathered from offset view
GMAX = 8               # chunks per dma_gather (1024 descs = SWDGE ring)
NEG = 0.2

F32 = mybir.dt.float32
BF16 = mybir.dt.bfloat16
FP8 = mybir.dt.float8e4
I32 = mybir.dt.int32
I16 = mybir.dt.int16
ALU = mybir.AluOpType
AF = mybir.ActivationFunctionType
NPBF = ml_dtypes.bfloat16
NPF8 = ml_dtypes.float8_e4m3


# ---------------------------------------------------------------- host prep
def _wrap16(flat):
    """dma_gather index layout: idx k -> [k%16, k//16], replicated x8."""
    n = flat.shape[0]
    assert n % 16 == 0
    w = flat.reshape(n // 16, 16).T          # [16, n//16]
    return np.tile(w, (8, 1))                 # [128, n//16]


def _prep(inputs):
    pos = np.ascontiguousarray(np.asarray(inputs["pos"], np.float32))
    z = np.asarray(inputs["z"]).astype(np.int64)
    ei = np.asarray(inputs["edge_index"]).astype(np.int64)
    batch = np.asarray(inputs["batch"]).astype(np.int64)

    loop = np.arange(N, dtype=np.int64)
    src = np.concatenate([ei[0], loop])
    dst = np.concatenate([ei[1], loop])
    order = np.argsort(dst, kind="stable")
    src_s = src[order].astype(np.int32)
    dst_s = dst[order].astype(np.int32)

    bnd = np.searchsorted(dst_s, np.arange(NB + 1) * P).astype(np.int64)

    block_lo = {}
    block_hi = {}
    for b in range(NB):
        e0, e1 = int(bnd[b]), int(bnd[b + 1])
        s = src_s[e0:e1]
        d = dst_s[e0:e1] - b * P
        m = s < LO
        block_lo[b] = (s[m], d[m])
        block_hi[b] = (s[~m] - LO, d[~m])

    nlo = np.zeros(NBPC, np.int64)
    nhi = np.zeros(NBPC, np.int64)
    for i in range(NBPC):
        for c in range(NCORES):
            b = NBPC * c + i
            nlo[i] = max(nlo[i], (len(block_lo[b][0]) + P - 1) // P)
            nhi[i] = max(nhi[i], (len(block_hi[b][0]) + P - 1) // P)
        if nlo[i] + nhi[i] == 0:
            nhi[i] = 1
    CH = nlo + nhi
    off = np.zeros(NBPC + 1, np.int64)
    off[1:] = np.cumsum(CH)
    TOTCH = int(off[-1])
    NCHMAX = int(CH.max())

    idx_w = np.zeros((NCORES, P, 8 * TOTCH), np.int16)
    dstrel = np.full((NCORES, P, TOTCH), -1.0, NPBF)
    oht = np.zeros((NCORES, P, TOTCH * P), NPF8)
    zidx = np.zeros((NCORES, P, NBPC), np.int32)
    batchrel = np.full((NCORES, P, NBPC), -1.0, np.float32)
    pos_shard = np.zeros((NCORES, SH, 3), np.float32)

    drng = np.arange(P)
    for c in range(NCORES):
        for i in range(NBPC):
            b = NBPC * c + i
            for (s, d), n_ch, cbase in (
                (block_lo[b], int(nlo[i]), int(off[i])),
                (block_hi[b], int(nhi[i]), int(off[i]) + int(nlo[i])),
            ):
                if n_ch == 0:
                    continue
                ne = len(s)
                flat = np.zeros(n_ch * P, np.int16)
                flat[:ne] = s.astype(np.int16)
                dr = np.full(n_ch * P, -1.0, np.float32)
                dr[:ne] = d.astype(np.float32)
                idx_w[c, :, 8 * cbase:8 * (cbase + n_ch)] = _wrap16(flat)
                drm = dr.reshape(n_ch, P)           # [chunk, e]
                dstrel[c, :, cbase:cbase + n_ch] = drm.T.astype(NPBF)
                oh = (drm[:, None, :] == drng[None, :, None])  # [c, d, e]
                oht[c, :, cbase * P:(cbase + n_ch) * P] = \
                    oh.transpose(1, 0, 2).reshape(P, n_ch * P).astype(NPF8)
        n0 = SH * c
        nodes = np.arange(n0, n0 + SH)
        valid = nodes < N
        zc = np.zeros(SH, np.int32)
        zc[valid] = z[nodes[valid]].astype(np.int32)
        zidx[c] = zc.reshape(NBPC, P).T
        bc = np.full(SH, -1.0, np.float32)
        bc[valid] = batch[nodes[valid]].astype(np.float32)
        batchrel[c] = bc.reshape(NBPC, P).T
        pos_shard[c][valid] = pos[nodes[valid]]

    counts = np.bincount(batch, minlength=G).astype(np.float32)
    cinv = (1.0 / np.maximum(counts, 1.0)).astype(np.float32).reshape(G, 1)

    iota_f = np.tile(np.arange(P, dtype=np.float32), (P, 1))
    # iota_c: [e, d*NCHMAX + c] = d  (c-innermost layout for batched MT build)
    iota_c = np.repeat(np.arange(P, dtype=np.float32), NCHMAX)[None, :]
    iota_c = np.tile(iota_c, (P, 1))

    def fold(Wn, asn, adn):
        Wf = np.asarray(inputs[Wn], np.float32)
        a_s = np.asarray(inputs[asn], np.float32)
        a_d = np.asarray(inputs[adn], np.float32)
        We = np.zeros((Wf.shape[0], WCOL), np.float32)
        We[:, 0:H] = Wf
        We[:, H] = Wf @ a_s
        We[:, H + 1] = Wf @ a_d
        return np.ascontiguousarray(We)

    consts = dict(
        iota=np.ascontiguousarray(iota_f),
        ident=np.eye(P, dtype=np.float32),
        iotac=np.ascontiguousarray(iota_c.astype(NPBF)),
        W1e=fold("W1", "a1_src", "a1_dst"),
        b1rep=np.ascontiguousarray(
            np.tile(np.asarray(inputs["b1"], np.float32), (P, 1))),
        W2e=fold("W2", "a2_src", "a2_dst"),
        b2rep=np.ascontiguousarray(
            np.tile(np.asarray(inputs["b2"], np.float32), (P, 1))),
        Wlin=np.ascontiguousarray(np.asarray(inputs["Wlin"], np.float32)),
        blinrep=np.ascontiguousarray(
            np.tile(np.asarray(inputs["blin"], np.float32), (P, 1))),
        emb=np.ascontiguousarray(np.asarray(inputs["emb"], np.float32)),
        cinv=cinv,
    )

    meta = dict(nlo=tuple(int(x) for x in nlo),
                nhi=tuple(int(x) for x in nhi),
                off=tuple(int(x) for x in off),
                TOTCH=TOTCH, NCHMAX=NCHMAX)
    percore = dict(idx_w=idx_w, dstrel=dstrel, oht=oht, zidx=zidx,
                   batchrel=batchrel, pos_shard=pos_shard)
    return meta, percore, consts


# ---------------------------------------------------------------- program
def _build(meta, analysis=False, gmax=GMAX, nqueues=None, gbufs=8,
           scratch=16384):
    if nqueues is None:
        nqueues = 1 if analysis else 4
    nlo = meta["nlo"]
    nhi = meta["nhi"]
    off = meta["off"]
    TOTCH = meta["TOTCH"]
    NCHMAX = meta["NCHMAX"]
    qstate = {"g": 0}

    def next_q():
        g = qstate["g"]
        qstate["g"] = g + 1
        return (g % gbufs) % nqueues

    nc = bacc.Bacc("TRN2", target_bir_lowering=False, debug=False,
                   enable_asserts=False,
                   num_devices=1 if analysis else NCORES,
                   num_swdge_queues=nqueues,
                   dynamic_dma_scratch_size=scratch)

    t_idx = nc.dram_tensor("idx_w", [P, 8 * TOTCH], I16, kind="ExternalInput")
    t_dstr = nc.dram_tensor("dstrel", [P, TOTCH], BF16, kind="ExternalInput")
    t_oht = nc.dram_tensor("oht", [P, TOTCH * P], FP8, kind="ExternalInput")
    t_zidx = nc.dram_tensor("zidx", [P, NBPC], I32, kind="ExternalInput")
    t_brel = nc.dram_tensor("batchrel", [P, NBPC], F32, kind="ExternalInput")
    t_pos = nc.dram_tensor("pos_shard", [SH, 3], F32, kind="ExternalInput")
    t_emb = nc.dram_tensor("emb", [T, 125], F32, kind="ExternalInput")
    t_iota = nc.dram_tensor("iota", [P, P], F32, kind="ExternalInput")
    t_ident = nc.dram_tensor("ident", [P, P], F32, kind="ExternalInput")
    t_iotac = nc.dram_tensor("iotac", [P, P * NCHMAX], BF16,
                             kind="ExternalInput")
    t_W1e = nc.dram_tensor("W1e", [H, WCOL], F32, kind="ExternalInput")
    t_b1rep = nc.dram_tensor("b1rep", [P, H], F32, kind="ExternalInput")
    t_W2e = nc.dram_tensor("W2e", [H, WCOL], F32, kind="ExternalInput")
    t_b2rep = nc.dram_tensor("b2rep", [P, H], F32, kind="ExternalInput")
    t_Wlin = nc.dram_tensor("Wlin", [H, O], F32, kind="ExternalInput")
    t_blinrep = nc.dram_tensor("blinrep", [P, O], F32, kind="ExternalInput")
    t_cinv = nc.dram_tensor("cinv", [G, 1], F32, kind="ExternalInput")
    t_out = nc.dram_tensor("out", [G, O], F32, kind="ExternalOutput")

    groups = [list(range(NCORES))]

    with tile.TileContext(nc) as tc:
        with (
            tc.tile_pool(name="const", bufs=1) as cpool,
            tc.tile_pool(name="work", bufs=3) as wpool,
            tc.tile_pool(name="gat", bufs=gbufs) as gpool,
            tc.tile_pool(name="blk", bufs=2) as bpool,
            tc.tile_pool(name="sml", bufs=4) as spool,
            tc.tile_pool(name="tp", bufs=1, space="PSUM") as tpool,
            tc.tile_pool(name="ps", bufs=4, space="PSUM") as pspool,
            tc.tile_pool(name="sdp", bufs=2, space="PSUM") as sdpool,
            tc.tile_pool(name="acc", bufs=1, space="PSUM") as apool,
            tc.tile_pool(name="dram", bufs=1, space="DRAM") as dpool,
        ):
            def cload(nm, t, shape, dtype=F32):
                tl = cpool.tile(shape, dtype, name=nm, tag=nm)
                nc.sync.dma_start(tl[:, :], t.ap())
                return tl

            iota_sb = cload("iota_sb", t_iota, [P, P])
            ident_sb = cload("ident_sb", t_ident, [P, P])
            iotac_sb = cload("iotac_sb", t_iotac, [P, P * NCHMAX], BF16)
            b1rep_sb = cload("b1rep_sb", t_b1rep, [P, H])
            b2rep_sb = cload("b2rep_sb", t_b2rep, [P, H])
            W1e_sb = cload("W1e_sb", t_W1e, [H, WCOL])
            W2e_sb = cload("W2e_sb", t_W2e, [H, WCOL])
            Wlin_sb = cload("Wlin_sb", t_Wlin, [H, O])
            blinrep_sb = cload("blinrep_sb", t_blinrep, [P, O])
            cinv_sb = cload("cinv_sb", t_cinv, [G, 1])
            idx_sb = cload("idx_sb", t_idx, [P, 8 * TOTCH], I16)
            dstr_sb = cload("dstr_sb", t_dstr, [P, TOTCH], BF16)
            zidx_sb = cload("zidx_sb", t_zidx, [P, NBPC], I32)
            brel_sb = cload("brel_sb", t_brel, [P, NBPC])

            # per-layer state kept in SBUF (overwritten between layers)
            sdst_all = cpool.tile([P, NBPC], BF16, name="sdst_all",
                                  tag="sdst_all")
            pe_all = cpool.tile([P, TOTCH], BF16, name="pe_all", tag="pe_all")

            hs1_sh = dpool.tile([SH, ROW], FP8, name="hs1_sh", tag="hs1_sh")
            hs1_f = dpool.tile([NPAD, ROW], FP8, name="hs1_f", tag="hs1_f",
                               addr_space="Shared")
            hs2_sh = dpool.tile([SH, ROW], FP8, name="hs2_sh", tag="hs2_sh")
            hs2_f = dpool.tile([NPAD, ROW], FP8, name="hs2_f", tag="hs2_f",
                               addr_space="Shared")
            pool_in = dpool.tile([G, O], F32, name="pool_in", tag="pool_in")
            pool_out = dpool.tile([G, O], F32, name="pool_out", tag="pool_out",
                                  addr_space="Shared")

            # store hs rows: [h fp8 | 1.0 fp8 | ssrc bf16] from f32 psum
            def store_hs(hs_ps, i, hs_sh):
                row = wpool.tile([P, ROW], FP8, name="row", tag="row")
                nc.scalar.activation(row[:, 0:H], hs_ps[:, 0:H], AF.Copy)
                nc.vector.memset(row[:, H:H + 1], 1.0)
                nc.vector.memset(row[:, H + 1:ROW], 0.0)
                rowb = row.bitcast(BF16)
                nc.scalar.activation(rowb[:, 65:66], hs_ps[:, H:H + 1],
                                     AF.Copy)
                nc.vector.tensor_scalar(out=sdst_all[:, i:i + 1],
                                        in0=hs_ps[:, H + 1:H + 2],
                                        scalar1=1.0, scalar2=None,
                                        op0=ALU.mult)
                nc.sync.dma_start(hs_sh[i * P:(i + 1) * P, :], row[:, :])

            # stage: x (f32 [node, 128]) -> transpose -> hs -> rows
            def stage_hs(i, x_sb, We_sb, hs_sh):
                xt_ps = tpool.tile([P, P], F32, name="xt_ps", tag="tps")
                nc.tensor.transpose(xt_ps[:, :], x_sb[:, :], ident_sb[:, :])
                xt = wpool.tile([P, P], F32, name="xt", tag="xt")
                nc.scalar.activation(xt[:, :], xt_ps[:, :], AF.Copy)
                hs_ps = pspool.tile([P, WCOL], F32, name="hs_ps", tag="hsps")
                nc.tensor.matmul(hs_ps[:, :], lhsT=xt[:, :], rhs=We_sb[:, :],
                                 start=True, stop=True)
                store_hs(hs_ps, i, hs_sh)

            # ---------------- layer 1 stage A ----------------
            for i in range(NBPC):
                x1 = wpool.tile([P, H], F32, name="x1", tag="x1")
                nc.gpsimd.indirect_dma_start(
                    out=x1[:, 3:128], out_offset=None, in_=t_emb.ap(),
                    in_offset=IndirectOffsetOnAxis(ap=zidx_sb[:, i:i + 1],
                                                   axis=0))
                nc.sync.dma_start(x1[:, 0:3], t_pos.ap()[i * P:(i + 1) * P, :])
                stage_hs(i, x1, W1e_sb, hs1_sh)

            if analysis:
                nc.sync.dma_start(hs1_f[0:SH, :], hs1_sh[:, :])
            else:
                nc.gpsimd.collective_compute(
                    "AllGather", ALU.bypass, groups,
                    ins=[hs1_sh[:, :]], outs=[hs1_f[:, :]])

            # ---------------- edge phase ----------------
            def edge_phase(post_block, hs_f):
                for i in range(NBPC):
                    nch = int(nlo[i]) + int(nhi[i])
                    o0 = int(off[i])
                    # static fp8 one-hot-transpose for the block
                    oht_sb = bpool.tile([P, NCHMAX * P], FP8, name="oht_sb",
                                        tag="oht_sb")
                    nc.sync.dma_start(
                        oht_sb[:, 0:nch * P],
                        t_oht.ap()[:, o0 * P:(o0 + nch) * P])
                    # SD[e, c] = sdst[dst(e, c)] via per-chunk PE matvec
                    sd_ps = sdpool.tile([P, NCHMAX], F32, name="sd_ps",
                                        tag="sdps")
                    for c in range(nch):
                        nc.tensor.matmul(
                            sd_ps[:, c:c + 1],
                            lhsT=oht_sb[:, c * P:(c + 1) * P],
                            rhs=sdst_all[:, i:i + 1],
                            start=True, stop=True)

                    halves = []
                    if nlo[i] > 0:
                        halves.append((int(nlo[i]), o0, hs_f[0:LO, :]))
                    if nhi[i] > 0:
                        halves.append((int(nhi[i]), o0 + int(nlo[i]),
                                       hs_f[LO:NPAD, :]))

                    # emit gathers, then per-group score pipeline
                    work = []
                    for n, cbase, table in halves:
                        for s0 in range(0, n, gmax):
                            sn = min(gmax, n - s0)
                            cb = cbase + s0
                            Gt = gpool.tile([P, gmax * ROW], FP8, name="Gt",
                                            tag="Gt")
                            nc.gpsimd.dma_gather(
                                out_ap=Gt[:, 0:sn * ROW].rearrange(
                                    "p (c s) -> p c s", s=ROW),
                                in_ap=table,
                                idxs_ap=idx_sb[:, 8 * cb:8 * (cb + sn)],
                                num_idxs=sn * P,
                                num_idxs_reg=sn * P,
                                elem_size=ROW,
                                queue_num=next_q(),
                            )
                            work.append((Gt, sn, cb))
                    for Gt, sn, cb in work:
                        # ssrc: bf16 at byte 130 of each 256B row
                        ssrc = Gt.bitcast(BF16).rearrange(
                            "p (c s) -> p c s", s=ROW // 2)[:, 0:sn, 65:66]
                        q = spool.tile([P, GMAX], F32, name="q", tag="q")
                        nc.vector.tensor_tensor(
                            out=q[:, 0:sn],
                            in0=sd_ps[:, cb - o0:cb - o0 + sn],
                            in1=ssrc.squeeze(2), op=ALU.add)
                        v = spool.tile([P, GMAX], F32, name="v", tag="v")
                        nc.vector.scalar_tensor_tensor(
                            out=v[:, 0:sn], in0=q[:, 0:sn], scalar=NEG,
                            in1=q[:, 0:sn], op0=ALU.mult, op1=ALU.max)
                        nc.scalar.activation(pe_all[:, cb:cb + sn],
                                             v[:, 0:sn], AF.Exp)

                    # batched MT build (c-innermost)
                    mt = bpool.tile([P, P * NCHMAX], BF16, name="mt", tag="mt")
                    mtv = mt.rearrange("p (d c) -> p d c", c=NCHMAX)
                    iov = iotac_sb.rearrange("p (d c) -> p d c", c=NCHMAX)
                    dstr_v = dstr_sb[:, o0:o0 + nch].unsqueeze(1) \
                        .to_broadcast([P, P, nch])
                    pe_v = pe_all[:, o0:o0 + nch].unsqueeze(1) \
                        .to_broadcast([P, P, nch])
                    nc.vector.tensor_tensor(
                        out=mtv[:, :, 0:nch], in0=iov[:, :, 0:nch],
                        in1=dstr_v, op=ALU.is_equal)
                    nc.vector.tensor_tensor(
                        out=mtv[:, :, 0:nch], in0=mtv[:, :, 0:nch],
                        in1=pe_v, op=ALU.mult)

                    # aggregation matmuls
                    num_ps = pspool.tile([P, 129], F32, name="num_ps",
                                         tag="hsps")
                    ci = 0
                    for Gt, sn, cb in work:
                        for j in range(sn):
                            lhsv = mtv[:, :, ci:ci + 1].squeeze(2)
                            nc.tensor.matmul(
                                num_ps[:, :], lhsT=lhsv,
                                rhs=Gt[:, j * ROW:j * ROW + 129],
                                start=(ci == 0), stop=(ci == nch - 1))
                            ci += 1
                    post_block(i, num_ps)

            # epilogue: x = elu(num/den + b)
            def finish_x(num_ps, brep_sb):
                den = spool.tile([P, 1], F32, name="den", tag="den")
                nc.vector.tensor_scalar(out=den[:, :], in0=num_ps[:, 128:129],
                                        scalar1=1e-30, scalar2=None,
                                        op0=ALU.max)
                rec = spool.tile([P, 1], F32, name="rec", tag="rec")
                nc.vector.reciprocal(rec[:, :], den[:, :])
                xp = wpool.tile([P, H], F32, name="xp", tag="xp")
                nc.vector.scalar_tensor_tensor(
                    out=xp[:, :], in0=num_ps[:, 0:128], scalar=rec[:, :],
                    in1=brep_sb[:, :], op0=ALU.mult, op1=ALU.add)
                xm = wpool.tile([P, H], F32, name="xm", tag="xm")
                nc.vector.tensor_scalar(out=xm[:, :], in0=xp[:, :],
                                        scalar1=0.0, scalar2=None, op0=ALU.min)
                xe = wpool.tile([P, H], F32, name="xe", tag="xe")
                nc.scalar.activation(xe[:, :], xm[:, :], AF.Exp)
                xr = wpool.tile([P, H], F32, name="xr", tag="xr")
                nc.vector.tensor_scalar(out=xr[:, :], in0=xp[:, :],
                                        scalar1=0.0, scalar2=None, op0=ALU.max)
                x2 = wpool.tile([P, H], F32, name="x2", tag="x2")
                nc.vector.scalar_tensor_tensor(
                    out=x2[:, :], in0=xe[:, :], scalar=-1.0, in1=xr[:, :],
                    op0=ALU.add, op1=ALU.add)
                return x2

            def post1(i, num_ps):
                x2 = finish_x(num_ps, b1rep_sb)
                stage_hs(i, x2, W2e_sb, hs2_sh)

            edge_phase(post1, hs1_f)

            if analysis:
                nc.sync.dma_start(hs2_f[0:SH, :], hs2_sh[:, :])
            else:
                nc.gpsimd.collective_compute(
                    "AllGather", ALU.bypass, groups,
                    ins=[hs2_sh[:, :]], outs=[hs2_f[:, :]])

            pool_ps = apool.tile([G, O], F32, name="pool_ps", tag="poolps")

            def post2(i, num_ps):
                x3 = finish_x(num_ps, b2rep_sb)
                xt_ps = tpool.tile([P, P], F32, name="x3t_ps", tag="tps")
                nc.tensor.transpose(xt_ps[:, :], x3[:, :], ident_sb[:, :])
                x3t = wpool.tile([P, P], F32, name="x3t", tag="xt")
                nc.scalar.activation(x3t[:, :], xt_ps[:, :], AF.Copy)
                y_ps = pspool.tile([P, O], F32, name="y_ps", tag="hsps")
                nc.tensor.matmul(y_ps[:, :], lhsT=x3t[:, :],
                                 rhs=Wlin_sb[:, :], start=True, stop=True)
                y_sb = wpool.tile([P, O], F32, name="y_sb", tag="y_sb")
                nc.vector.tensor_tensor(out=y_sb[:, :], in0=y_ps[:, :],
                                        in1=blinrep_sb[:, :], op=ALU.add)
                Mg = wpool.tile([P, G], F32, name="Mg", tag="Mg")
                nc.vector.tensor_scalar(out=Mg[:, :], in0=iota_sb[:, 0:G],
                                        scalar1=brel_sb[:, i:i + 1],
                                        scalar2=None, op0=ALU.is_equal)
                nc.tensor.matmul(pool_ps[:, :], lhsT=Mg[:, :], rhs=y_sb[:, :],
                                 start=(i == 0), stop=(i == NBPC - 1))

            edge_phase(post2, hs2_f)

            # ---------------- final reduce ----------------
            pool_sb = spool.tile([G, O], F32, name="pool_sb", tag="pool_sb")
            nc.scalar.activation(pool_sb[:, :], pool_ps[:, :], AF.Copy)
            nc.sync.dma_start(pool_in[:, :], pool_sb[:, :])
            if analysis:
                nc.sync.dma_start(pool_out[:, :], pool_in[:, :])
            else:
                nc.gpsimd.collective_compute(
                    "AllReduce", ALU.add, groups,
                    ins=[pool_in[:, :]], outs=[pool_out[:, :]])
            red_sb = spool.tile([G, O], F32, name="red_sb", tag="red_sb")
            nc.sync.dma_start(red_sb[:, :], pool_out[:, :])
            fin_sb = spool.tile([G, O], F32, name="fin_sb", tag="fin_sb")
            nc.vector.tensor_scalar(out=fin_sb[:, :], in0=red_sb[:, :],
                                    scalar1=cinv_sb[:, :], scalar2=None,
                                    op0=ALU.mult)
            nc.sync.dma_start(t_out.ap(), fin_sb[:, :])

    nc.compile()
    nc.m = get_hw_module(nc.m)
    return nc


_CACHE = {}


def _get_nc(meta, analysis=False):
    key = (meta["TOTCH"], meta["nlo"], meta["nhi"], analysis)
    if key not in _CACHE:
        _CACHE[key] = _build(meta, analysis=analysis)
    return _CACHE[key]


def run(inputs, trace=False, analysis=False, **kw):
    meta, percore, consts = _prep(inputs)
    nc = _get_nc(meta, analysis=analysis)
    in_maps = []
    for c in range(NCORES):
        m = dict(consts)
        for k in ("idx_w", "dstrel", "oht", "zidx", "batchrel", "pos_shard"):
            m[k] = np.ascontiguousarray(percore[k][c])
        in_maps.append(m)
    if analysis:
        return nc, in_maps
    res = bass_utils.run_bass_kernel_spmd(
        nc, in_maps, core_ids=list(range(NCORES)), trace=trace, **kw)
    return res


def kernel(**inputs):
    res = run(inputs, trace=False)
    return res.results[0]["out"]
